# revision 45
# baseline (speedup 1.0000x reference)
"""Trainium2 Bass kernel for nn_LorentzGraphConvolution.

Row-sharded across 8 NeuronCores: core c owns rows [c*1536, (c+1)*1536) of
the attention matrix / output. Every core redundantly computes the tiny
linear phase (h, k for all N; q for its local rows) from broadcast inputs,
so no collectives are needed; the only large input is each core's
[12288, 1536] fp8 slab of adj^T (host-transposed, host-cast).

Layout strategy (per core):
  - Linear phases run TRANSPOSED (features on partitions): one stationary
    weight load + 512-col moving matmuls.  Per-node normalization scalars
    are computed on the NATURAL side after an xbar DMA-transpose, batched
    into [128, T] tiles; the Lorentz scaling is applied naturally
    (per-partition rows, free-broadcast scale) and a second xbar produces
    the transposed consumable (hT for the k-matmul; kT/qmT for attention).
    Work is spread across engines: psum evacuation on ACT, squares on
    GpSimd, math chain on DVE, xbars split across the SP/ACT DMA queues.
  - Phases are emitted interleaved (A0 Aq A1 Q A2 B0 B1 B2) in column
    segments so phase C's per-segment tile dependencies let attention
    start while later segments still cook.
  - att is computed TRANSPOSED (attT[j, i], j on partitions) via
    matmul(lhsT=kT block, rhs=qmT chunk) row-packed pairs; the adjacency
    mask accumulates BIG*adjT into the same PSUM bank via one stationary
    fp8 DoubleRow matmul per j-tile (adjT streamed from HBM as fp8 in the
    row-interleaved DoubleRow layout; 4x less HBM traffic than f32, and
    the mask matmul runs at 0.5 cycles/row).
  - sigmoid(+mask bias) runs once per [128, 1024] double-bank PSUM tile.
"""

import math
import os
import sys
from contextlib import ExitStack

for _p in ("/opt/trn_rl_repo", "/root/.axon_site/_ro/trn_rl_repo", "/root/.axon_site"):
    if os.path.isdir(_p) and _p not in sys.path:
        sys.path.insert(0, _p)

import ml_dtypes
import numpy as np

import concourse.bass as bass
import concourse.tile as tile
from concourse import bacc, bass_utils, masks, mybir
from concourse.tile import add_dep_helper

DT = mybir.dt
F32 = DT.float32
BF16 = DT.bfloat16
FP8 = DT.float8e4
AF = mybir.ActivationFunctionType
ALU = mybir.AluOpType

N_FULL = 12288
D = 64
N_CORES = 8
R_FULL = N_FULL // N_CORES  # 1536 rows per core
SEG = 4096                  # linear-phase column segment


def emit(tc, io, nn, rr, esc, esc_q, esc_k, sig_scale, sig_bias):
    nc = tc.nc
    TJ = nn // 128          # 96 global 128-row tiles
    IC = 512                # attention column block (i-chunk)
    NIC = rr // IC          # 3
    JG = 8                  # j-tiles per adjT load group
    NG = TJ // JG
    NSEG = nn // SEG        # 3
    I32 = DT.int32

    ctx = ExitStack()

    const = ctx.enter_context(tc.tile_pool(name="const", bufs=1))
    persist = ctx.enter_context(tc.tile_pool(name="persist", bufs=1))
    slab = ctx.enter_context(tc.tile_pool(name="slab", bufs=2))
    psum_big = ctx.enter_context(tc.tile_pool(name="psum_big", bufs=2, space="PSUM"))
    psum_sup = ctx.enter_context(tc.tile_pool(name="psum_sup", bufs=2, space="PSUM"))
    small = ctx.enter_context(tc.tile_pool(name="small", bufs=2))
    sqwork = ctx.enter_context(tc.tile_pool(name="sqwork", bufs=2))
    strip_pool = ctx.enter_context(tc.tile_pool(name="strips", bufs=3))
    sig_pool = ctx.enter_context(tc.tile_pool(name="sig", bufs=3))
    out_pool = ctx.enter_context(tc.tile_pool(name="outp", bufs=2))

    # ---- constants / small inputs -------------------------------------
    xT_s = persist.tile([65, nn], BF16)
    nc.sync.dma_start(xT_s[:], io["xT"][:])
    xqT_s = const.tile([65, rr], BF16)
    nc.sync.dma_start(xqT_s[:], io["xqT"][:])
    wT_s = const.tile([65, 64], BF16)
    nc.sync.dma_start(wT_s[:], io["wT"][:])
    wqT_s = const.tile([65, 64], BF16)
    nc.sync.dma_start(wqT_s[:], io["wqT"][:])
    wkT_s = const.tile([65, 64], BF16)
    nc.sync.dma_start(wkT_s[:], io["wkT"][:])
    ident = const.tile([64, 64], F32)
    masks.make_identity(nc, ident[:])
    sig_bias_ap = const.tile([128, 1], F32)
    nc.vector.memset(sig_bias_ap[:], sig_bias)
    magic = const.tile([128, 1], I32)
    nc.vector.memset(magic[:], 0x5F3759DF)

    def fast_rsqrt(dst, x, nb, tag, iters=1):
        """dst = 1/sqrt(x) via bit-trick + Newton iterations (DVE only)."""
        xi = x.bitcast(I32)
        sh = small.tile([128, nb], I32, tag=tag + "sh")
        nc.vector.tensor_scalar(sh[:], xi, 1, None, ALU.arith_shift_right)
        y0 = small.tile([128, nb], F32, tag=tag + "y0")
        nc.vector.tensor_tensor(y0.bitcast(I32)[:], magic[:].to_broadcast((128, nb)),
                                sh[:], ALU.subtract)
        y = y0[:]
        for it in range(iters):
            ysq = small.tile([128, nb], F32, tag=tag + "ysq")
            nc.vector.tensor_tensor(ysq[:], y, y, ALU.mult)
            t = small.tile([128, nb], F32, tag=tag + "t")
            nc.vector.tensor_tensor(t[:], ysq[:], x, ALU.mult)
            w = small.tile([128, nb], F32, tag=tag + "w")
            nc.vector.tensor_scalar(w[:], t[:], -0.5, 1.5, ALU.mult, ALU.add)
            if it == iters - 1:
                nc.vector.tensor_tensor(dst, y, w[:], ALU.mult)
            else:
                yn = small.tile([128, nb], F32, tag=tag + "yn")
                nc.vector.tensor_tensor(yn[:], y, w[:], ALU.mult)
                y = yn[:]

    # ---- Lorentz linear segment (transposed in, natural out) ----------
    def lorentz_seg(w_ap, rhs_ap, scols, esc_, neg, raw, nat, outT, alt):
        """One column segment of a Lorentz-linear phase.

        raw:  [128, scols] scratch; row 64 must hold ones if outT feeds a
              later matmul as [0:65] rhs (bias row).
        nat:  [128, scols] receives the natural layout, scaled in place.
        outT: [128, scols] receives the transposed final (2nd xbar).
        """
        T = scols // 128
        nat3 = nat.rearrange("p (t n) -> p t n", n=128)
        for u in range((scols + 1023) // 1024):
            c0 = u * 1024
            cw = min(1024, scols - c0)
            ps = psum_big.tile([128, 1024], F32, tag="big")
            for v in range(cw // 512):
                nc.tensor.matmul(ps[0:64, v * 512:(v + 1) * 512], w_ap,
                                 rhs_ap[:, c0 + v * 512:c0 + (v + 1) * 512],
                                 start=True, stop=True)
            if (u + alt) % 2 == 0:
                nc.scalar.copy(raw[0:64, c0:c0 + cw], ps[0:64, 0:cw])
            else:
                nc.vector.tensor_copy(raw[0:64, c0:c0 + cw], ps[0:64, 0:cw])
            # narrow xbar: only rows 0:79 carry data (row 64 = bias ones,
            # 65:79 zeros); nat cols 80:127 hold zeros from one-time memset
            nc.sync.dma_start(nat3[:, u * 8:u * 8 + cw // 128, 0:80],
                              raw[0:80, c0:c0 + cw], transpose=True)
        # per-node scalars over this segment's T tiles
        sg = small.tile([128, T], F32, tag="sg")
        nc.scalar.activation(sg[:], nat3[:, :, 0], AF.Sigmoid)
        ssq_bf = small.tile([128, T], BF16, tag="ssqb")
        for g0 in range(0, T, 16):
            gw = min(16, T - g0)
            sq = sqwork.tile([128, 16 * 63], BF16, tag="sq")
            sq3 = sq.rearrange("p (t d) -> p t d", d=63)[:, 0:gw, :]
            v = nat3[:, g0:g0 + gw, 1:64]
            nc.gpsimd.tensor_tensor(sq3, v, v, ALU.mult)
            with nc.allow_low_precision(reason="63-term bf16 sq-sum, 2e-2 tol"):
                nc.vector.tensor_reduce(ssq_bf[:, g0:g0 + gw], sq3,
                                        axis=mybir.AxisListType.X, op=ALU.add)
        ssq = small.tile([128, T], F32, tag="ssq")
        nc.vector.tensor_scalar_max(ssq[:], ssq_bf[:], 1e-8)
        timew = small.tile([128, T], F32, tag="tw")
        nc.vector.tensor_scalar(timew[:], sg[:], esc_, 1.1, ALU.mult, ALU.add)
        t2 = small.tile([128, T], F32, tag="t2")
        nc.vector.tensor_tensor(t2[:], timew[:], timew[:], ALU.mult)
        t2m1 = small.tile([128, T], F32, tag="t2m1")
        nc.vector.tensor_scalar_add(t2m1[:], t2[:], -1.0)
        r1 = small.tile([128, T], F32, tag="r1")
        fast_rsqrt(r1[:], t2m1[:], T, "q1")
        r2 = small.tile([128, T], F32, tag="r2")
        fast_rsqrt(r2[:], ssq[:], T, "q2")
        sq1 = small.tile([128, T], F32, tag="sq1")
        nc.vector.tensor_tensor(sq1[:], t2m1[:], r1[:], ALU.mult)
        sqs_bf = small.tile([128, T], BF16, tag="sqsb")
        nc.vector.tensor_tensor(sqs_bf[:], sq1[:], r2[:], ALU.mult)
        time_bf = small.tile([128, T], BF16, tag="timb")
        if neg:
            nc.vector.tensor_scalar_mul(time_bf[:], timew[:], -1.0)
        else:
            nc.vector.tensor_copy(time_bf[:], timew[:])
        # natural-side assembly + 2nd xbar, in halves for pipelining
        outT3 = outT.rearrange("p (t n) -> p t n", n=128)
        H = (T + 1) // 2
        for hh in range(2):
            lo, hi = hh * H, min(T, (hh + 1) * H)
            if lo >= hi:
                continue
            nv = nat3[:, lo:hi, :]
            nc.vector.tensor_tensor(nv[:, :, 1:64], nv[:, :, 1:64],
                                    sqs_bf[:, lo:hi].to_broadcast(
                                        (128, hi - lo, 63)), ALU.mult)
            nc.vector.tensor_copy(nv[:, :, 0], time_bf[:, lo:hi])
            nc.sync.dma_start(outT3[:, lo:hi, :], nat[:, lo * 128:hi * 128],
                              transpose=True)

    # ---- linear phases, interleaved for overlap -----------------------
    # A: h for all rows -> hpad segs (natural) + hT segs (transposed)
    # Aq: h for local rows (from xqT) -> hqT ; Q: qm -> qmT_full
    # B: k for all rows -> kT_stk stacked-pair segs
    hpad_seg = [None] * NSEG
    hT_seg = [None] * NSEG
    kT_seg = [None] * NSEG

    # persistent raw scratch slabs (preset once; row 64 = ones bias row)
    rawAB = [persist.tile([128, SEG], BF16, tag=f"raw{i}", name=f"raw{i}")
             for i in range(2)]
    rawQQ = [persist.tile([128, rr], BF16, tag=f"qraw{i}", name=f"qraw{i}")
             for i in range(2)]
    for t, w in (rawAB[0], SEG), (rawAB[1], SEG), (rawQQ[0], rr), (rawQQ[1], rr):
        nc.gpsimd.memset(t[64:128, 0:w], 0.0)
        nc.gpsimd.memset(t[64:65, 0:w], 1.0)

    natK = [persist.tile([128, SEG], BF16, tag=f"knat{i}", name=f"knat{i}")
            for i in range(2)]
    natQ2 = [persist.tile([128, rr], BF16, tag=f"qnat{i}", name=f"qnat{i}")
             for i in range(2)]

    def phase_A(s):
        nat = persist.tile([128, SEG], BF16, tag=f"hpad{s}")
        nc.gpsimd.memset(nat[:], 0.0)
        hT = persist.tile([128, SEG], BF16, tag=f"hT{s}")
        lorentz_seg(wT_s[:], xT_s[:, s * SEG:(s + 1) * SEG], SEG, esc, False,
                    rawAB[s % 2], nat, hT, s)
        hpad_seg[s] = nat
        hT_seg[s] = hT

    def phase_B(s):
        nat = natK[s % 2]
        if s < 2:
            nc.gpsimd.memset(nat[:], 0.0)
        kflat = slab.tile([128, SEG], BF16, tag="kflat")
        lorentz_seg(wkT_s[:], hT_seg[s][0:65, :], SEG, esc_k, False,
                    rawAB[s % 2], nat, kflat, s)
        kstk = persist.tile([128, SEG // 2], BF16, tag=f"kT{s}")
        kf3 = kflat.rearrange("p (t two n) -> p t two n", two=2, n=128)
        ks3 = kstk.rearrange("p (t n) -> p t n", n=128)
        nc.sync.dma_start(ks3[0:64], kf3[0:64, :, 0, :])
        nc.sync.dma_start(ks3[64:128], kf3[0:64, :, 1, :])
        kT_seg[s] = kstk

    phase_A(0)

    nc.gpsimd.memset(natQ2[0][:], 0.0)
    hqT = persist.tile([128, rr], BF16, tag="hqT")
    lorentz_seg(wT_s[:], xqT_s[:], rr, esc, False, rawQQ[0], natQ2[0], hqT, 0)

    phase_B(0)

    nc.gpsimd.memset(natQ2[1][:], 0.0)
    qmT_full = persist.tile([128, rr], BF16)
    lorentz_seg(wqT_s[:], hqT[0:65, :], rr, esc_q, True, rawQQ[1], natQ2[1],
                qmT_full, 1)
    nc.sync.dma_start(qmT_full[64:128, :], qmT_full[0:64, :])

    phase_A(1)
    phase_B(1)
    phase_A(2)
    phase_B(2)

    # ---- phase C: attention + support --------------------------------
    adjT = io["adjT"]
    adjT4 = adjT.rearrange("(g t p) i -> g t p i", t=JG, p=128)
    PPS = SEG // 256        # 16 pair-blocks per kT segment
    TPS = SEG // 128        # 32 j-tiles per hpad segment
    for c in range(NIC):
        supT = psum_sup.tile([64, IC], F32, tag="supT")
        prev_sup = None
        pend = None  # (sig2, tp) whose MM3s are deferred one pair

        def emit_mm3(sig2, tp):
            nonlocal prev_sup
            for jj in range(2):
                j = 2 * tp + jj
                seg = (j * 128) // SEG
                jl = j - seg * TPS
                mm_s = nc.tensor.matmul(
                    supT[:], hpad_seg[seg][:, jl * 128:jl * 128 + 64],
                    sig2[:, jj * 512:(jj + 1) * 512],
                    start=(j == 0), stop=(j == TJ - 1))
                if prev_sup is not None:
                    add_dep_helper(mm_s.ins, prev_sup.ins, sync=False,
                                   reason="supT accum order")
                prev_sup = mm_s

        for g in range(NG):
            st = strip_pool.tile([128, JG * IC], BF16, tag="strip")
            st3 = st.rearrange("p (t i) -> p t i", i=IC)
            nc.sync.dma_start(
                st3[:], adjT4[g, :, :, c * IC:(c + 1) * IC].rearrange(
                    "t p i -> p t i"))
            for tl in range(JG // 2):
                tp = g * (JG // 2) + tl
                seg = (2 * tp * 128) // SEG
                tpl = tp - seg * PPS
                att2 = psum_big.tile([128, 1024], F32, tag="big")
                qch = slice(c * IC, (c + 1) * IC)
                nc.tensor.matmul(att2[:, 0:512],
                                 kT_seg[seg][0:64,
                                             tpl * 128:(tpl + 1) * 128],
                                 qmT_full[0:64, qch],
                                 start=True, stop=True,
                                 tile_position=(0, 0))
                nc.tensor.matmul(att2[:, 512:1024],
                                 kT_seg[seg][64:128,
                                             tpl * 128:(tpl + 1) * 128],
                                 qmT_full[64:128, qch],
                                 start=True, stop=True,
                                 tile_position=(64, 0))
                sig2 = sig_pool.tile([128, 1024], BF16, tag="sig")
                nc.scalar.activation(sig2[:], att2[:], AF.Sigmoid,
                                     bias=sig_bias_ap[:], scale=sig_scale)
                masked = sig_pool.tile([128, 1024], BF16, tag="masked")
                nc.vector.tensor_tensor(
                    masked.rearrange("p (t i) -> p t i", i=IC)[:],
                    sig2.rearrange("p (t i) -> p t i", i=IC)[:],
                    st3[:, 2 * tl:2 * tl + 2, :], ALU.mult)
                if pend is not None:
                    emit_mm3(*pend)
                pend = (masked, tp)
        emit_mm3(*pend)
        # normalize + write out this i-chunk (batched over its 4 row tiles)
        supTs = out_pool.tile([64, IC], F32, tag="supTs", bufs=1)
        nc.vector.tensor_copy(supTs[:], supT[:])
        NT = IC // 128
        o4 = out_pool.tile([128, NT * 64], F32, tag="o4")
        o43 = o4.rearrange("p (s d) -> p s d", d=64)
        for s in range(NT):
            supn = psum_sup.tile([128, 64], F32, tag="supn")
            nc.tensor.transpose(supn[:], supTs[:, s * 128:(s + 1) * 128],
                                ident[:])
            nc.vector.tensor_copy(o43[:, s, :], supn[:])
        sq4 = out_pool.tile([128, NT * 64], F32, tag="sq4")
        sq43 = sq4.rearrange("p (s d) -> p s d", d=64)
        nc.scalar.activation(sq4[:], o4[:], AF.Square)
        tot = small.tile([128, NT], F32, tag="ftot")
        nc.vector.tensor_reduce(tot[:], sq43[:], axis=mybir.AxisListType.X,
                                op=ALU.add)
        inner = small.tile([128, NT], F32, tag="finner")
        nc.vector.scalar_tensor_tensor(inner[:], sq43[:, :, 0], -2.0,
                                       tot[:], ALU.mult, ALU.add)
        negv = small.tile([128, NT], F32, tag="fneg")
        nc.vector.tensor_scalar_mul(negv[:], inner[:], -1.0)
        absv = small.tile([128, NT], F32, tag="fabs")
        nc.vector.tensor_tensor(absv[:], inner[:], negv[:], ALU.max)
        clipv = small.tile([128, NT], F32, tag="fclip")
        nc.vector.tensor_scalar_max(clipv[:], absv[:], 1e-8)
        rs = small.tile([128, NT], F32, tag="frs")
        fast_rsqrt(rs[:], clipv[:], NT, "fq", iters=2)
        oall = out_pool.tile([128, NT * 64], F32, tag="oall")
        oall3 = oall.rearrange("p (s d) -> p s d", d=64)
        nc.vector.tensor_tensor(oall3[:], o43[:],
                                rs[:].to_broadcast((128, NT, 64)), ALU.mult)
        for s in range(NT):
            r0 = c * IC + s * 128
            nc.sync.dma_start(io["out"][r0:r0 + 128, :], oall3[:, s, :])

    ctx.close()


def build(nn, rr, esc, esc_q, esc_k, sig_scale, sig_bias, num_devices=N_CORES):
    nc = bacc.Bacc("TRN2", target_bir_lowering=False, debug=False,
                   num_devices=num_devices)
    io = {
        "adjT": nc.dram_tensor("adjT", [nn, rr], BF16, kind="ExternalInput").ap(),
        "xT": nc.dram_tensor("xT", [65, nn], BF16, kind="ExternalInput").ap(),
        "xqT": nc.dram_tensor("xqT", [65, rr], BF16, kind="ExternalInput").ap(),
        "wT": nc.dram_tensor("wT", [65, 64], BF16, kind="ExternalInput").ap(),
        "wqT": nc.dram_tensor("wqT", [65, 64], BF16, kind="ExternalInput").ap(),
        "wkT": nc.dram_tensor("wkT", [65, 64], BF16, kind="ExternalInput").ap(),
        "out": nc.dram_tensor("out", [rr, 64], F32, kind="ExternalOutput").ap(),
    }
    with tile.TileContext(nc) as tc:
        emit(tc, io, nn, rr, esc, esc_q, esc_k, sig_scale, sig_bias)
    nc.compile()
    return nc


def make_in_maps(inputs, nn, rr, n_cores):
    bf = ml_dtypes.bfloat16
    f8 = ml_dtypes.float8_e4m3
    x = np.asarray(inputs["x"], np.float32)
    adj = np.ascontiguousarray(np.asarray(inputs["adj"], np.float32))
    W = np.asarray(inputs["W"], np.float32)
    b = np.asarray(inputs["b"], np.float32)
    Wq = np.asarray(inputs["Wq"], np.float32)
    bq = np.asarray(inputs["bq"], np.float32)
    Wk = np.asarray(inputs["Wk"], np.float32)
    bk = np.asarray(inputs["bk"], np.float32)

    xT_ext = np.concatenate([x.T, np.ones((1, nn), np.float32)], 0).astype(bf)
    wT_ext = np.concatenate([W.T, b[None, :]], 0).astype(bf)
    wqT_ext = np.concatenate([Wq.T, bq[None, :]], 0).astype(bf)
    wkT_ext = np.concatenate([Wk.T, bk[None, :]], 0).astype(bf)

    adjb = adj.astype(bf)
    in_maps = []
    for c in range(n_cores):
        r0 = c * rr
        in_maps.append({
            "adjT": np.ascontiguousarray(adjb[r0:r0 + rr].T),
            "xT": np.ascontiguousarray(xT_ext),
            "xqT": np.ascontiguousarray(xT_ext[:, r0:r0 + rr]),
            "wT": wT_ext,
            "wqT": wqT_ext,
            "wkT": wkT_ext,
        })
    return in_maps


def consts_from_inputs(inputs):
    scale = float(np.asarray(inputs["scale"], np.float32))
    scale_q = float(np.asarray(inputs["scale_q"], np.float32))
    scale_k = float(np.asarray(inputs["scale_k"], np.float32))
    att_bias = float(np.asarray(inputs["att_bias"], np.float32))
    att_scale = float(np.asarray(inputs["att_scale"], np.float32))
    esc = math.exp(scale)
    esc_q = math.exp(scale_q)
    esc_k = math.exp(scale_k)
    sig_scale = 2.0 / att_scale
    sig_bias = 2.0 / att_scale + att_bias
    return esc, esc_q, esc_k, sig_scale, sig_bias


def kernel(**inputs):
    nn, rr = N_FULL, R_FULL
    consts = consts_from_inputs(inputs)
    nc = build(nn, rr, *consts)
    in_maps = make_in_maps(inputs, nn, rr, N_CORES)
    res = bass_utils.run_bass_kernel_spmd(nc, in_maps,
                                          core_ids=list(range(N_CORES)))
    return np.concatenate([res.results[c]["out"] for c in range(N_CORES)],
                          axis=0)
